# revision 43
# baseline (speedup 1.0000x reference)
"""Causal multi-head attention (B=1, S=4096, H=16 heads x 64, hidden 1024) on
8 Trainium2 NeuronCores.

Sharding: tensor-parallel over heads, 2 heads per core. Each core receives the
full activation (pre-transposed to [hidden, S] layout), its 128-row slice of
wq/wk/wv (transposed) and 128-column slice of wo (transposed), computes
q/k/v projections + flash-style causal attention for its 2 heads, applies its
slice of the output projection, and writes a full-shape partial output. The
host sums the 8 partials (the TP all-reduce) to produce the final output.

Kernel layout notes:
  - scores are computed TRANSPOSED: ST[sk, sq] = kT_tile^T @ qT_block, so the
    softmax numerator exp() runs PSUM->SBUF on the scalar engine with no
    transposes of the probability matrix anywhere.
  - the softmax denominator comes for free from the PV matmul by augmenting
    v with a ones column (stationary operand [v | 1], M=65): output row 64
    accumulates sum_k exp(s).
  - normalization (1/l = exp(-ln l) on the scalar engine) is applied per
    head on OT eviction, before heads are mixed by the out-projection.
  - matmul operands are float16 (11-bit mantissa ~ TF32 accuracy): 2-byte
    weight loads pipeline behind the matmul stream and keep the PE's HAM
    clock gate warm (4-byte fp32/fp32r weight loads serialize, halving
    effective clock). PSUM accumulation is fp32 throughout.
  - emission is a software-pipelined work queue: each attention slot emits
    scores (ST), exp, mask, the previous slot's PV, plus one drip-fed chunk
    of deferred work (previous block's normalize/out-proj, next block's
    projections) so no engine ever stalls at block boundaries.
"""
import sys
sys.path.insert(0, "/opt/trn_rl_repo")

import numpy as np

import concourse.bass as bass
import concourse.mybir as mybir
import concourse.tile as tile
from concourse.bass_utils import run_bass_kernel_spmd

# ---------------------------------------------------------------- constants
S = 4096          # sequence length
HID = 1024        # hidden dim
NCORES = 8
HPC = 2           # heads per core
HD = 64           # head dim
EPC = HPC * HD    # 128 e-dims (head-concat) per core
SB = 512          # q-block width
NB = S // SB      # 8 q-blocks
NT = S // 128     # 32 k-tiles
GROUP = 2         # k-tiles per exp batch (2 psum banks)

F32 = mybir.dt.float32
F32R = mybir.dt.float32r
F16 = mybir.dt.float16
DT = F16  # matmul operand dtype

_MAX_WAITS = 1    # this walrus build allows a single sync-wait per instruction


def _split_waits(nc):
    """Hoist extra sync-waits onto inserted same-engine drain carriers."""
    n = 0
    for fn in nc.m.functions:
        for bb in fn.blocks:
            insts = bb.instructions
            i = 0
            while i < len(insts):
                inst = insts[i]
                si = inst.sync_info
                w = list(si.on_wait) if si is not None and si.on_wait else []
                if len(w) > _MAX_WAITS:
                    chunks = [w[j:j + _MAX_WAITS] for j in range(0, len(w), _MAX_WAITS)]
                    si.on_wait = chunks[-1]
                    for ch in chunks[:-1]:
                        # EventSemaphore carrier: pure wait, no pipeline flush
                        # (InstDrain stalls the engine pipe ~1.5us per use).
                        d = mybir.InstEventSemaphore(
                            name=f"{inst.name}_ws{n}", ins=[], outs=[])
                        d.engine = inst.engine
                        d.sync_info = mybir.SyncInfo(on_wait=ch, on_update=[])
                        insts.insert(i, d)
                        i += 1
                        n += 1
                i += 1
    return n


def _build_nc():
    nc = bass.Bass(target_bir_lowering=False)

    xT = nc.declare_dram_parameter("xT", [HID, S], DT, isOutput=False)
    wqT = nc.declare_dram_parameter("wqT", [HID, EPC], DT, isOutput=False)
    wkT = nc.declare_dram_parameter("wkT", [HID, EPC], DT, isOutput=False)
    wvT = nc.declare_dram_parameter("wvT", [HID, EPC], DT, isOutput=False)
    woT = nc.declare_dram_parameter("woT", [EPC, HID], DT, isOutput=False)
    cmask = nc.declare_dram_parameter("cmask", [128, 4 * SB], DT, isOutput=False)
    ones = nc.declare_dram_parameter("ones", [1, 128], DT, isOutput=False)
    ident = nc.declare_dram_parameter("ident", [128, 128], DT, isOutput=False)
    out = nc.declare_dram_parameter("out", [S, HID], F32, isOutput=True)

    KH = HID // 128  # 8 contraction chunks for projections

    with tile.TileContext(nc) as tc:
        with tc.tile_pool(name="const", bufs=1) as const, \
             tc.tile_pool(name="qk", bufs=1) as qk, \
             tc.tile_pool(name="xt", bufs=3) as xtp, \
             tc.tile_pool(name="vt", bufs=2) as vtp, \
             tc.tile_pool(name="pt", bufs=6) as ptp, \
             tc.tile_pool(name="att", bufs=2) as attp, \
             tc.tile_pool(name="osb", bufs=3) as osbp, \
             tc.tile_pool(name="rl", bufs=4) as rlp, \
             tc.tile_pool(name="bc", bufs=2) as bcp, \
             tc.tile_pool(name="ps", bufs=3, space="PSUM") as psp, \
             tc.tile_pool(name="ot", bufs=2, space="PSUM") as otp:

            # ---- constants / weights. Load order matters at startup: the
            # first projection only needs xt(0) (emitted just below) + wq;
            # everything else loads behind them and hides under compute.
            # Less critical loads ride the gpsimd queues.
            wq_sb = const.tile([128, KH, EPC], DT, tag="wq")
            wk_sb = const.tile([128, KH, EPC], DT, tag="wk")
            wv_sb = const.tile([128, KH, EPC], DT, tag="wv")
            wo_sb = const.tile([EPC, HID], DT, tag="wo")
            cm_sb = const.tile([128, 4 * SB], DT, tag="cm")
            ones_sb = const.tile([1, 128], DT, tag="ones")
            id_sb = const.tile([128, 128], DT, tag="id")
            qT = qk.tile([128, S], DT, tag="qT")   # [e(2 heads), s]
            kT = qk.tile([128, S], DT, tag="kT")
            vbuf = qk.tile([128, HPC, NT, 65], DT, tag="v")  # [sk, h, t, v|1]

            def load_consts_front():
                nc.sync.dma_start(
                    out=wq_sb, in_=wqT.rearrange("(k p) m -> p k m", p=128))

            def load_consts_rest():
                for w_d, w_s in ((wkT, wk_sb), (wvT, wv_sb)):
                    nc.sync.dma_start(
                        out=w_s, in_=w_d.rearrange("(k p) m -> p k m", p=128))
                nc.gpsimd.dma_start(out=id_sb, in_=ident[:, :])
                nc.gpsimd.dma_start(out=cm_sb, in_=cmask[:, :])
                nc.gpsimd.dma_start(out=ones_sb, in_=ones[:, :])
                nc.gpsimd.dma_start(out=wo_sb, in_=woT[:, :])
                nc.vector.memset(
                    vbuf.rearrange("p a b c -> p (a b c)"), 1.0)

            def make_norm_outproj(b, ots):
                """Closures for block b's normalize + out-projection, emitted
                later (interleaved into the next block's attention slots) so
                the reciprocal/eviction latency hides under PE work."""
                att = attp.tile([128, SB], DT, tag="att", name=f"att{b}")

                bcps = psp.tile([128, SB], F32, tag="st", name=f"bc{b}")
                bc = bcp.tile([128, SB], F32, tag="bc", name=f"bcs{b}")

                def bcast_head(h):
                    # l row PSUM->SBUF on DVE, then broadcast across the
                    # head's 64 partitions on the PE
                    lrow = rlp.tile([1, SB], DT, tag="rl", name=f"rl{b}_{h}")
                    nc.vector.tensor_copy(out=lrow, in_=ots[h][64:65, :])
                    nc.tensor.matmul(bcps[64 * h:64 * (h + 1), :],
                                     ones_sb[:, 0:64], lrow,
                                     start=True, stop=True)

                def recip_muls():
                    # 1/l = exp(-ln l) on the scalar engine: much lower latency
                    # than the DVE reciprocal and off the DVE queue entirely
                    # (Log and Exp share the natural_log_exp table set)
                    t1 = bcp.tile([128, SB], F32, tag="bc", name=f"lnl{b}")
                    nc.scalar.activation(out=t1, in_=bcps,
                                         func=mybir.ActivationFunctionType.Ln)
                    nc.scalar.activation(out=bc, in_=t1,
                                         func=mybir.ActivationFunctionType.Exp,
                                         scale=-1.0)
                    for h in range(HPC):
                        nc.vector.tensor_mul(att[64 * h:64 * (h + 1), :],
                                             ots[h][0:64, :], bc[64 * h:64 * (h + 1), :])

                def outproj(mlist):
                    for m in mlist:
                        osb = osbp.tile([128, HID], F32, tag="osb",
                                        name=f"osb{b}_{m}")
                        for n2 in range(2):
                            op = psp.tile([128, 512], F32, tag="st",
                                          name=f"op{b}_{m}_{n2}")
                            nc.tensor.matmul(op, att[:, m * 128:(m + 1) * 128],
                                             wo_sb[:, n2 * 512:(n2 + 1) * 512],
                                             start=True, stop=True)
                            nc.vector.tensor_copy(
                                out=osb[:, n2 * 512:(n2 + 1) * 512], in_=op)
                        r0 = (4 * b + m) * 128
                        # stores ride the (otherwise idle) GpSimd SWDGE queues
                        # so they never head-block the xt loads on nc.sync
                        nc.gpsimd.dma_start(out=out[r0:r0 + 128, :], in_=osb)

                return [lambda: bcast_head(0), lambda: bcast_head(1),
                        recip_muls,
                        lambda: outproj([0, 1]), lambda: outproj([2, 3])]

            def load_xt(bb):
                xt = xtp.tile([128, KH, SB], DT, tag="xt", name=f"xt{bb}")
                src_ap = xT.rearrange("(k p) s -> p k s", p=128)
                if bb == 0:
                    # split the first load so the first projection matmuls
                    # (k-chunks 0..3) can start after half the transfer
                    nc.sync.dma_start(out=xt[:, 0:KH // 2, :],
                                      in_=src_ap[:, 0:KH // 2, 0:SB])
                    nc.sync.dma_start(out=xt[:, KH // 2:, :],
                                      in_=src_ap[:, KH // 2:, 0:SB])
                else:
                    nc.sync.dma_start(
                        out=xt, in_=src_ap[:, :, bb * SB:(bb + 1) * SB])
                return xt

            xts = {0: load_xt(0)}
            load_consts_front()
            load_consts_rest()

            def make_proj_chunks(bb):
                """Projection work for block bb as slot-sized closures."""
                slb = slice(bb * SB, (bb + 1) * SB)
                holder = {}

                def c_pref():
                    if bb + 1 < NB:
                        xts[bb + 1] = load_xt(bb + 1)

                def c_w(w_sb, dst):
                    def run():
                        ps = psp.tile([128, SB], F32, tag="st",
                                      name=f"ps{bb}_{dst.name}")
                        for k in range(KH):
                            nc.tensor.matmul(ps, w_sb[:, k, :], xts[bb][:, k, :],
                                             start=(k == 0), stop=(k == KH - 1))
                        nc.vector.tensor_copy(out=dst[:, slb], in_=ps)
                    return run

                def c_v():
                    ps = psp.tile([128, SB], F32, tag="st", name=f"psv{bb}")
                    for k in range(KH):
                        nc.tensor.matmul(ps, wv_sb[:, k, :], xts[bb][:, k, :],
                                         start=(k == 0), stop=(k == KH - 1))
                    vt = vtp.tile([128, SB], DT, tag="vt", name=f"vt{bb}")
                    nc.vector.tensor_copy(out=vt, in_=ps)
                    holder["vt"] = vt

                def c_flips():
                    vt = holder["vt"]
                    fps = []

                    def evict_flip(j, fp):
                        t = 4 * bb + j
                        nc.vector.tensor_copy(out=vbuf[:, 0, t, 0:64],
                                              in_=fp[:, 0:64])
                        nc.vector.tensor_copy(out=vbuf[:, 1, t, 0:64],
                                              in_=fp[:, 64:128])

                    for j in range(4):
                        fp = psp.tile([128, 128], F32, tag="st",
                                      name=f"fp{bb}_{j}")
                        nc.tensor.matmul(fp, vt[:, j * 128:(j + 1) * 128], id_sb,
                                         start=True, stop=True)
                        fps.append((j, fp))
                        if len(fps) > 1:
                            evict_flip(*fps.pop(0))
                    evict_flip(*fps.pop(0))

                return [c_pref, c_w(wq_sb, qT), c_w(wk_sb, kT), c_v, c_flips]

            for c in make_proj_chunks(0):   # bootstrap block 0 inline
                c()
            prev_no = []    # norm/out-proj closures of the previous block
            for b in range(NB):
                sl = slice(b * SB, (b + 1) * SB)
                # work queue for this block's attention slots: the previous
                # block's normalize/out-proj + the next block's projections
                pj = make_proj_chunks(b + 1) if b + 1 < NB else []
                # two proj chunks of PE work separate the reciprocal from the
                # out-projections that consume its result
                no, deferred = list(prev_no), list(prev_no[0:3])
                order = [pj, pj, pj, no[3:4], pj, no[4:5], pj]
                for lst in order:
                    if lst:
                        deferred.append(lst.pop(0))

                # ---------- attention for q-block b (both heads)
                # software pipeline over (head, group) slots: emit ST(slot)
                # then PV(slot-1), so PV never stalls the PE on the exp.
                # The previous block's deferred normalize/out-proj closures are
                # drip-fed between slots so their latency hides under PE work.
                ntl = 4 * (b + 1)  # causal k-tiles
                ots = [otp.tile([65, SB], F32, tag="ot", name=f"ot{b}_{h}")
                       for h in range(HPC)]
                groups = [list(range(g, min(g + GROUP, ntl)))
                          for g in range(0, ntl, GROUP)]
                slots = [(h, grp) for grp in groups for h in range(HPC)]
                pend = []   # pending (h, grp, pt) awaiting PV emission

                def emit_pv(h, grp, pt):
                    for i, t in enumerate(grp):
                        off = 128 * (t - 4 * b) if t >= 4 * b else 0
                        nc.tensor.matmul(
                            ots[h][:, off:], vbuf[:, h, t, :],
                            pt[:, i * SB + off:(i + 1) * SB],
                            start=(t == 0), stop=(t == ntl - 1),
                            skip_group_check=True)

                for h, grp in slots:
                    hsl = slice(64 * h, 64 * (h + 1))
                    st = psp.tile([128, GROUP * SB], F32, tag="st",
                                  name=f"st{b}_{h}_{grp[0]}")
                    for i, t in enumerate(grp):
                        # diagonal tiles only need queries >= 128j (causal)
                        off = 128 * (t - 4 * b) if t >= 4 * b else 0
                        nc.tensor.matmul(
                            st[:, i * SB + off:(i + 1) * SB],
                            kT[hsl, t * 128:(t + 1) * 128],
                            qT[hsl, b * SB + off:(b + 1) * SB],
                            start=True, stop=True)
                    L = len(grp) * SB
                    pt = ptp.tile([128, GROUP * SB], DT, tag="pt",
                                  name=f"pt{b}_{h}_{grp[0]}")
                    nc.scalar.activation(out=pt[:, :L], in_=st[:, :L],
                                         func=mybir.ActivationFunctionType.Exp,
                                         scale=float(HD) ** -0.5)
                    for i, t in enumerate(grp):
                        j = t - 4 * b
                        if j >= 0:  # triangle mask on the 128-wide diagonal
                            psl = slice(i * SB + 128 * j, i * SB + 128 * (j + 1))
                            nc.vector.tensor_mul(pt[:, psl], pt[:, psl],
                                                 cm_sb[:, 0:128])
                    pend.append((h, grp, pt))
                    if len(pend) > 2:   # PV lags 2 slots; exp+mask fully hidden
                        emit_pv(*pend.pop(0))
                    if deferred:
                        deferred.pop(0)()
                while pend:
                    emit_pv(*pend.pop(0))
                for work in deferred:   # flush leftovers (early small blocks)
                    work()
                prev_no = make_norm_outproj(b, ots)

            for work in prev_no:
                work()

    _split_waits(nc)
    return nc


_cached = {}


def _get_nc():
    if "nc" not in _cached:
        _cached["nc"] = _build_nc()
    return _cached["nc"]


def make_in_maps(x, wq, wk, wv, wo):
    x = np.asarray(x, dtype=np.float32)
    wq, wk, wv, wo = (np.asarray(a, dtype=np.float32) for a in (wq, wk, wv, wo))
    B = x.shape[0]
    assert x.shape == (B, S, HID)

    dt = np.float16
    xT = np.ascontiguousarray(x[0].T.astype(dt))            # [HID, S]
    # static causal masks for the 4 diagonal tile offsets
    p = np.arange(128)[:, None]
    i = np.arange(SB)[None, :]
    cm = np.concatenate([(p + 128 * j <= i) for j in range(4)],
                        axis=1).astype(dt)                  # [128, 4*SB]
    ones = np.ones((1, 128), dtype=dt)
    ident = np.eye(128, dtype=dt)

    in_maps = []
    for c in range(NCORES):
        esl = slice(c * EPC, (c + 1) * EPC)
        in_maps.append({
            "xT": xT,
            "wqT": np.ascontiguousarray(wq[esl, :].T.astype(dt)),
            "wkT": np.ascontiguousarray(wk[esl, :].T.astype(dt)),
            "wvT": np.ascontiguousarray(wv[esl, :].T.astype(dt)),
            "woT": np.ascontiguousarray(wo[:, esl].T.astype(dt)),
            "cmask": cm,
            "ones": ones,
            "ident": ident,
        })
    return in_maps


def kernel(x, wq, wk, wv, wo):
    B = np.asarray(x).shape[0]
    in_maps = make_in_maps(x, wq, wk, wv, wo)
    nc = _get_nc()
    res = run_bass_kernel_spmd(nc, in_maps, core_ids=list(range(NCORES)))
    acc = res.results[0]["out"].astype(np.float32)
    for c in range(1, NCORES):
        acc = acc + res.results[c]["out"]
    return acc.reshape(B, S, HID)


if __name__ == "__main__":
    # smoke test against numpy reference
    rng = np.random.default_rng(0)
    x = rng.standard_normal((1, S, HID), dtype=np.float32)
    lim = float(np.sqrt(6.0 / (HID + 16 * HD)))
    wq, wk, wv, wo = (rng.uniform(-lim, lim, (1024, 1024)).astype(np.float32)
                      for _ in range(4))
    got = kernel(x=x, wq=wq, wk=wk, wv=wv, wo=wo)
    print("kernel output", got.shape, got.dtype, got.flat[:4])


# revision 44
# speedup vs baseline: 1.2375x; 1.2375x over previous
"""Causal multi-head attention (B=1, S=4096, H=16 heads x 64, hidden 1024) on
8 Trainium2 NeuronCores.

Sharding: tensor-parallel over heads, 2 heads per core. Each core receives the
full activation (pre-transposed to [hidden, S] layout), its 128-row slice of
wq/wk/wv (transposed) and 128-column slice of wo (transposed), computes
q/k/v projections + flash-style causal attention for its 2 heads, applies its
slice of the output projection, and writes a full-shape partial output. The
host sums the 8 partials (the TP all-reduce) to produce the final output.

Kernel layout notes:
  - scores are computed TRANSPOSED: ST[sk, sq] = kT_tile^T @ qT_block, so the
    softmax numerator exp() runs PSUM->SBUF on the scalar engine with no
    transposes of the probability matrix anywhere.
  - the softmax denominator comes for free from the PV matmul by augmenting
    v with a ones column (stationary operand [v | 1], M=65): output row 64
    accumulates sum_k exp(s).
  - normalization (1/l = exp(-ln l) on the scalar engine) is applied per
    head on OT eviction, before heads are mixed by the out-projection.
  - matmul operands are float16 (11-bit mantissa ~ TF32 accuracy): 2-byte
    weight loads pipeline behind the matmul stream and keep the PE's HAM
    clock gate warm (4-byte fp32/fp32r weight loads serialize, halving
    effective clock). PSUM accumulation is fp32 throughout.
  - emission is a software-pipelined work queue: each attention slot emits
    scores (ST), exp, mask, the previous slot's PV, plus one drip-fed chunk
    of deferred work (previous block's normalize/out-proj, next block's
    projections) so no engine ever stalls at block boundaries.
"""
import sys
sys.path.insert(0, "/opt/trn_rl_repo")

import numpy as np

import concourse.bass as bass
import concourse.mybir as mybir
import concourse.tile as tile
from concourse.bass_utils import run_bass_kernel_spmd

# ---------------------------------------------------------------- constants
S = 4096          # sequence length
HID = 1024        # hidden dim
NCORES = 8
HPC = 2           # heads per core
HD = 64           # head dim
EPC = HPC * HD    # 128 e-dims (head-concat) per core
SB = 512          # q-block width
NB = S // SB      # 8 q-blocks
NT = S // 128     # 32 k-tiles
GROUP = 2         # k-tiles per exp batch (2 psum banks)

F32 = mybir.dt.float32
F32R = mybir.dt.float32r
F16 = mybir.dt.float16
DT = F16  # matmul operand dtype

_MAX_WAITS = 1    # this walrus build allows a single sync-wait per instruction


def _split_waits(nc):
    """Hoist extra sync-waits onto inserted same-engine drain carriers."""
    n = 0
    for fn in nc.m.functions:
        for bb in fn.blocks:
            insts = bb.instructions
            i = 0
            while i < len(insts):
                inst = insts[i]
                si = inst.sync_info
                w = list(si.on_wait) if si is not None and si.on_wait else []
                if len(w) > _MAX_WAITS:
                    chunks = [w[j:j + _MAX_WAITS] for j in range(0, len(w), _MAX_WAITS)]
                    si.on_wait = chunks[-1]
                    for ch in chunks[:-1]:
                        # EventSemaphore carrier: pure wait, no pipeline flush
                        # (InstDrain stalls the engine pipe ~1.5us per use).
                        d = mybir.InstEventSemaphore(
                            name=f"{inst.name}_ws{n}", ins=[], outs=[])
                        d.engine = inst.engine
                        d.sync_info = mybir.SyncInfo(on_wait=ch, on_update=[])
                        insts.insert(i, d)
                        i += 1
                        n += 1
                i += 1
    return n


def _build_nc():
    nc = bass.Bass(target_bir_lowering=False)

    xT = nc.declare_dram_parameter("xT", [HID, S], DT, isOutput=False)
    wqT = nc.declare_dram_parameter("wqT", [HID, EPC], DT, isOutput=False)
    wkT = nc.declare_dram_parameter("wkT", [HID, EPC], DT, isOutput=False)
    wvT = nc.declare_dram_parameter("wvT", [HID, EPC], DT, isOutput=False)
    woT = nc.declare_dram_parameter("woT", [EPC, HID], DT, isOutput=False)
    cmask = nc.declare_dram_parameter("cmask", [128, 4 * SB], DT, isOutput=False)
    ones = nc.declare_dram_parameter("ones", [1, 128], DT, isOutput=False)
    ident = nc.declare_dram_parameter("ident", [128, 128], DT, isOutput=False)
    out = nc.declare_dram_parameter("out", [S, HID], F32, isOutput=True)

    KH = HID // 128  # 8 contraction chunks for projections

    with tile.TileContext(nc) as tc:
        with tc.tile_pool(name="const", bufs=1) as const, \
             tc.tile_pool(name="qk", bufs=1) as qk, \
             tc.tile_pool(name="xt", bufs=3) as xtp, \
             tc.tile_pool(name="vt", bufs=2) as vtp, \
             tc.tile_pool(name="pt", bufs=4) as ptp, \
             tc.tile_pool(name="att", bufs=2) as attp, \
             tc.tile_pool(name="osb", bufs=3) as osbp, \
             tc.tile_pool(name="rl", bufs=4) as rlp, \
             tc.tile_pool(name="bc", bufs=2) as bcp, \
             tc.tile_pool(name="ps", bufs=3, space="PSUM") as psp, \
             tc.tile_pool(name="ot", bufs=2, space="PSUM") as otp:

            # ---- constants / weights. Load order matters at startup: the
            # first projection only needs xt(0) (emitted just below) + wq;
            # everything else loads behind them and hides under compute.
            # Less critical loads ride the gpsimd queues.
            wq_sb = const.tile([128, KH, EPC], DT, tag="wq")
            wk_sb = const.tile([128, KH, EPC], DT, tag="wk")
            wv_sb = const.tile([128, KH, EPC], DT, tag="wv")
            wo_sb = const.tile([EPC, HID], DT, tag="wo")
            cm_sb = const.tile([128, 4 * SB], DT, tag="cm")
            ones_sb = const.tile([1, 128], DT, tag="ones")
            id_sb = const.tile([128, 128], DT, tag="id")
            qT = qk.tile([128, S], DT, tag="qT")   # [e(2 heads), s]
            kT = qk.tile([128, S], DT, tag="kT")
            vbuf = qk.tile([128, HPC, NT, 65], DT, tag="v")  # [sk, h, t, v|1]

            def load_consts_front():
                nc.sync.dma_start(
                    out=wq_sb, in_=wqT.rearrange("(k p) m -> p k m", p=128))

            def load_consts_rest():
                for w_d, w_s in ((wkT, wk_sb), (wvT, wv_sb)):
                    nc.sync.dma_start(
                        out=w_s, in_=w_d.rearrange("(k p) m -> p k m", p=128))
                nc.gpsimd.dma_start(out=id_sb, in_=ident[:, :])
                nc.gpsimd.dma_start(out=cm_sb, in_=cmask[:, :])
                nc.gpsimd.dma_start(out=ones_sb, in_=ones[:, :])
                nc.gpsimd.dma_start(out=wo_sb, in_=woT[:, :])
                nc.vector.memset(
                    vbuf.rearrange("p a b c -> p (a b c)"), 1.0)

            def make_norm_outproj(b, ots):
                """Closures for block b's normalize + out-projection, emitted
                later (interleaved into the next block's attention slots) so
                the reciprocal/eviction latency hides under PE work."""
                att = attp.tile([128, SB], DT, tag="att", name=f"att{b}")

                bcps = psp.tile([128, SB], F32, tag="st", name=f"bc{b}")
                bc = bcp.tile([128, SB], F32, tag="bc", name=f"bcs{b}")

                def bcast_head(h):
                    # l row PSUM->SBUF on DVE, then broadcast across the
                    # head's 64 partitions on the PE
                    lrow = rlp.tile([1, SB], DT, tag="rl", name=f"rl{b}_{h}")
                    nc.vector.tensor_copy(out=lrow, in_=ots[h][64:65, :])
                    nc.tensor.matmul(bcps[64 * h:64 * (h + 1), :],
                                     ones_sb[:, 0:64], lrow,
                                     start=True, stop=True)

                def recip_muls():
                    # 1/l = exp(-ln l) on the scalar engine: much lower latency
                    # than the DVE reciprocal and off the DVE queue entirely
                    # (Log and Exp share the natural_log_exp table set)
                    t1 = bcp.tile([128, SB], F32, tag="bc", name=f"lnl{b}")
                    nc.scalar.activation(out=t1, in_=bcps,
                                         func=mybir.ActivationFunctionType.Ln)
                    nc.scalar.activation(out=bc, in_=t1,
                                         func=mybir.ActivationFunctionType.Exp,
                                         scale=-1.0)
                    for h in range(HPC):
                        nc.vector.tensor_mul(att[64 * h:64 * (h + 1), :],
                                             ots[h][0:64, :], bc[64 * h:64 * (h + 1), :])

                def outproj(mlist):
                    for m in mlist:
                        osb = osbp.tile([128, HID], F32, tag="osb",
                                        name=f"osb{b}_{m}")
                        for n2 in range(2):
                            op = psp.tile([128, 512], F32, tag="st",
                                          name=f"op{b}_{m}_{n2}")
                            nc.tensor.matmul(op, att[:, m * 128:(m + 1) * 128],
                                             wo_sb[:, n2 * 512:(n2 + 1) * 512],
                                             start=True, stop=True)
                            nc.vector.tensor_copy(
                                out=osb[:, n2 * 512:(n2 + 1) * 512], in_=op)
                        r0 = (4 * b + m) * 128
                        # stores ride the (otherwise idle) GpSimd SWDGE queues
                        # so they never head-block the xt loads on nc.sync
                        nc.gpsimd.dma_start(out=out[r0:r0 + 128, :], in_=osb)

                return [lambda: bcast_head(0), lambda: bcast_head(1),
                        recip_muls,
                        lambda: outproj([0, 1]), lambda: outproj([2, 3])]

            def load_xt(bb):
                xt = xtp.tile([128, KH, SB], DT, tag="xt", name=f"xt{bb}")
                src_ap = xT.rearrange("(k p) s -> p k s", p=128)
                if bb == 0:
                    # split the first load so the first projection matmuls
                    # (k-chunks 0..3) can start after half the transfer
                    nc.sync.dma_start(out=xt[:, 0:KH // 2, :],
                                      in_=src_ap[:, 0:KH // 2, 0:SB])
                    nc.sync.dma_start(out=xt[:, KH // 2:, :],
                                      in_=src_ap[:, KH // 2:, 0:SB])
                else:
                    nc.sync.dma_start(
                        out=xt, in_=src_ap[:, :, bb * SB:(bb + 1) * SB])
                return xt

            xts = {0: load_xt(0)}
            load_consts_front()
            load_consts_rest()

            def make_proj_chunks(bb):
                """Projection work for block bb as slot-sized closures."""
                slb = slice(bb * SB, (bb + 1) * SB)
                holder = {}

                def c_pref():
                    if bb + 1 < NB:
                        xts[bb + 1] = load_xt(bb + 1)

                def c_w(w_sb, dst):
                    def run():
                        ps = psp.tile([128, SB], F32, tag="st",
                                      name=f"ps{bb}_{dst.name}")
                        for k in range(KH):
                            nc.tensor.matmul(ps, w_sb[:, k, :], xts[bb][:, k, :],
                                             start=(k == 0), stop=(k == KH - 1))
                        nc.vector.tensor_copy(out=dst[:, slb], in_=ps)
                    return run

                def c_v():
                    ps = psp.tile([128, SB], F32, tag="st", name=f"psv{bb}")
                    for k in range(KH):
                        nc.tensor.matmul(ps, wv_sb[:, k, :], xts[bb][:, k, :],
                                         start=(k == 0), stop=(k == KH - 1))
                    vt = vtp.tile([128, SB], DT, tag="vt", name=f"vt{bb}")
                    nc.vector.tensor_copy(out=vt, in_=ps)
                    holder["vt"] = vt

                def c_flips():
                    vt = holder["vt"]
                    fps = []

                    def evict_flip(j, fp):
                        t = 4 * bb + j
                        nc.vector.tensor_copy(out=vbuf[:, 0, t, 0:64],
                                              in_=fp[:, 0:64])
                        nc.vector.tensor_copy(out=vbuf[:, 1, t, 0:64],
                                              in_=fp[:, 64:128])

                    for j in range(4):
                        fp = psp.tile([128, 128], F32, tag="st",
                                      name=f"fp{bb}_{j}")
                        nc.tensor.matmul(fp, vt[:, j * 128:(j + 1) * 128], id_sb,
                                         start=True, stop=True)
                        fps.append((j, fp))
                        if len(fps) > 1:
                            evict_flip(*fps.pop(0))
                    evict_flip(*fps.pop(0))

                return [c_pref, c_w(wq_sb, qT), c_w(wk_sb, kT), c_v, c_flips]

            for c in make_proj_chunks(0):   # bootstrap block 0 inline
                c()
            prev_no = []    # norm/out-proj closures of the previous block
            for b in range(NB):
                sl = slice(b * SB, (b + 1) * SB)
                # work queue for this block's attention slots: the previous
                # block's normalize/out-proj + the next block's projections
                pj = make_proj_chunks(b + 1) if b + 1 < NB else []
                # two proj chunks of PE work separate the reciprocal from the
                # out-projections that consume its result
                no, deferred = list(prev_no), list(prev_no[0:3])
                order = [pj, pj, pj, no[3:4], pj, no[4:5], pj]
                for lst in order:
                    if lst:
                        deferred.append(lst.pop(0))

                # ---------- attention for q-block b (both heads)
                # software pipeline over (head, group) slots: emit ST(slot)
                # then PV(slot-1), so PV never stalls the PE on the exp.
                # The previous block's deferred normalize/out-proj closures are
                # drip-fed between slots so their latency hides under PE work.
                ntl = 4 * (b + 1)  # causal k-tiles
                ots = [otp.tile([65, SB], F32, tag="ot", name=f"ot{b}_{h}")
                       for h in range(HPC)]
                groups = [list(range(g, min(g + GROUP, ntl)))
                          for g in range(0, ntl, GROUP)]
                slots = [(h, grp) for grp in groups for h in range(HPC)]
                pend = []   # pending (h, grp, pt) awaiting PV emission

                def emit_pv(h, grp, pt):
                    for i, t in enumerate(grp):
                        off = 128 * (t - 4 * b) if t >= 4 * b else 0
                        nc.tensor.matmul(
                            ots[h][:, off:], vbuf[:, h, t, :],
                            pt[:, i * SB + off:(i + 1) * SB],
                            start=(t == 0), stop=(t == ntl - 1),
                            skip_group_check=True)

                for h, grp in slots:
                    hsl = slice(64 * h, 64 * (h + 1))
                    st = psp.tile([128, GROUP * SB], F32, tag="st",
                                  name=f"st{b}_{h}_{grp[0]}")
                    for i, t in enumerate(grp):
                        # diagonal tiles only need queries >= 128j (causal)
                        off = 128 * (t - 4 * b) if t >= 4 * b else 0
                        nc.tensor.matmul(
                            st[:, i * SB + off:(i + 1) * SB],
                            kT[hsl, t * 128:(t + 1) * 128],
                            qT[hsl, b * SB + off:(b + 1) * SB],
                            start=True, stop=True)
                    L = len(grp) * SB
                    pt = ptp.tile([128, GROUP * SB], DT, tag="pt",
                                  name=f"pt{b}_{h}_{grp[0]}")
                    nc.scalar.activation(out=pt[:, :L], in_=st[:, :L],
                                         func=mybir.ActivationFunctionType.Exp,
                                         scale=float(HD) ** -0.5)
                    for i, t in enumerate(grp):
                        j = t - 4 * b
                        if j >= 0:  # triangle mask on the 128-wide diagonal
                            psl = slice(i * SB + 128 * j, i * SB + 128 * (j + 1))
                            nc.vector.tensor_mul(pt[:, psl], pt[:, psl],
                                                 cm_sb[:, 0:128])
                    pend.append((h, grp, pt))
                    if len(pend) > 1:
                        emit_pv(*pend.pop(0))
                    if deferred:
                        deferred.pop(0)()
                while pend:
                    emit_pv(*pend.pop(0))
                for work in deferred:   # flush leftovers (early small blocks)
                    work()
                prev_no = make_norm_outproj(b, ots)

            for work in prev_no:
                work()

    _split_waits(nc)
    return nc


_cached = {}


def _get_nc():
    if "nc" not in _cached:
        _cached["nc"] = _build_nc()
    return _cached["nc"]


def make_in_maps(x, wq, wk, wv, wo):
    x = np.asarray(x, dtype=np.float32)
    wq, wk, wv, wo = (np.asarray(a, dtype=np.float32) for a in (wq, wk, wv, wo))
    B = x.shape[0]
    assert x.shape == (B, S, HID)

    dt = np.float16
    xT = np.ascontiguousarray(x[0].T.astype(dt))            # [HID, S]
    # static causal masks for the 4 diagonal tile offsets
    p = np.arange(128)[:, None]
    i = np.arange(SB)[None, :]
    cm = np.concatenate([(p + 128 * j <= i) for j in range(4)],
                        axis=1).astype(dt)                  # [128, 4*SB]
    ones = np.ones((1, 128), dtype=dt)
    ident = np.eye(128, dtype=dt)

    in_maps = []
    for c in range(NCORES):
        esl = slice(c * EPC, (c + 1) * EPC)
        in_maps.append({
            "xT": xT,
            "wqT": np.ascontiguousarray(wq[esl, :].T.astype(dt)),
            "wkT": np.ascontiguousarray(wk[esl, :].T.astype(dt)),
            "wvT": np.ascontiguousarray(wv[esl, :].T.astype(dt)),
            "woT": np.ascontiguousarray(wo[:, esl].T.astype(dt)),
            "cmask": cm,
            "ones": ones,
            "ident": ident,
        })
    return in_maps


def kernel(x, wq, wk, wv, wo):
    B = np.asarray(x).shape[0]
    in_maps = make_in_maps(x, wq, wk, wv, wo)
    nc = _get_nc()
    res = run_bass_kernel_spmd(nc, in_maps, core_ids=list(range(NCORES)))
    acc = res.results[0]["out"].astype(np.float32)
    for c in range(1, NCORES):
        acc = acc + res.results[c]["out"]
    return acc.reshape(B, S, HID)


if __name__ == "__main__":
    # smoke test against numpy reference
    rng = np.random.default_rng(0)
    x = rng.standard_normal((1, S, HID), dtype=np.float32)
    lim = float(np.sqrt(6.0 / (HID + 16 * HD)))
    wq, wk, wv, wo = (rng.uniform(-lim, lim, (1024, 1024)).astype(np.float32)
                      for _ in range(4))
    got = kernel(x=x, wq=wq, wk=wk, wv=wv, wo=wo)
    print("kernel output", got.shape, got.dtype, got.flat[:4])
